# revision 25
# baseline (speedup 1.0000x reference)
"""Trainium2 Bass kernel for a Matching Network attention head.

Reference computation (see problem statement):
    q_proj = query @ W1[:D]                       # [Q, D]
    s_proj = support @ W1[D:]                     # [S, D]
    hidden = relu(q_proj[:,None,:] + s_proj[None,:,:] + b1)   # [Q, S, D]
    scores = einsum('qsd,d->qs', hidden, W2) + b2
    weights = softmax(scores, axis=1)
    logits  = weights @ onehot(support_labels)    # [Q, n_way]

Sharding strategy (8 cores): shard the SUPPORT set (40 of 320 rows per
core) and replicate the queries.  Each core produces the *unnormalized*
partial numerators and denominator of the softmax-weighted average:

    part[w, q]  = sum_{s in shard} exp(score[s,q]) * onehot[s,w]   (w < 20)
    part[20, q] = sum_{s in shard} exp(score[s,q])

The host sums the partials over cores and divides - softmax over the
full support set falls out exactly (b2 is a constant shift over s and
cancels in the softmax, so it is dropped).  exp() is computed without a
max-subtraction: scores are ~N(0, 0.7) for this problem so fp32 exp is
safe and exact.

Per-core device program:
  - one const "blob" DMA + two queryT DMAs (bf16, host-prepped layout)
  - qpT[dout, q]  = W1a^T @ queryT      (PE, bf16, fp32 psum)
  - spbT[dout, s] = W1b^T @ supportT + b1 (b1 folded in as a K=1 matmul)
  - For each s: H = relu(qpT + spbT[:, s]) as a single fused
    tensor_scalar(add, max) on DVE (bf16 in/out -> 4x mode, ~749 ns) or
    an activation(Relu, bias) on ACT (~1990 ns); 58/22 split so both
    engines finish together.
  - scores[s, q] = sum_d W2[d] * H[d, q] via one-hot-column matmuls:
    lhsT is [128, 32] with W2's d-block in column (s//4), output goes to
    psum partitions [32*(s%4) .. +32).  tile_position=(0, 32*j) makes 4
    consecutive matmuls run concurrently in distinct 32-column groups of
    the PE array.  s is split into two halves with separate psum bank
    sets so exp/matmul of the first half overlaps the second half.
  - E = exp(scores) on ACT (psum -> sbuf, bf16)
  - part += [onehot_half | ones_half]^T @ E on PE, copied out [21, Q].
"""

import numpy as np
import ml_dtypes

bf16 = ml_dtypes.bfloat16

N_CORES = 8
Q, D, S, NWAY = 2048, 256, 320, 20
SP = S // N_CORES          # 40 support rows per core
NQC = 4                    # q chunks of 512 (one psum bank each)
QC = Q // NQC
NR = SP // 4               # 10 rounds of 4 concurrent s-values
NRH = NR // 2              # rounds per half (5)

# const-blob column layout (bf16, [128, NB])
OFF_W1A = 0                # [128, 256] x2 (din block major)
OFF_W1B = 512
OFF_ST = 1024              # [128, 40] x2
OFF_W2C = 1104             # [128, 640]: 2 dblk x 10 rounds x [128, 32]
OFF_OHM = 1744             # [128, 21] x2 (ohmA | ohmB)
OFF_B1 = 1786              # [row0 = b1, 256 cols]
NB = 2042

_compiled = None


def _build_nc():
    import concourse.tile as tile
    from concourse import mybir
    from concourse.bacc import Bacc

    f32 = mybir.dt.float32
    b16 = mybir.dt.bfloat16
    RELU = mybir.ActivationFunctionType.Relu
    EXP = mybir.ActivationFunctionType.Exp
    ADD = mybir.AluOpType.add
    MAX = mybir.AluOpType.max

    nc = Bacc()
    blob_d = nc.declare_dram_parameter("blob", [128, NB], b16, isOutput=False)
    qT_d = nc.declare_dram_parameter("qT", [D, Q], b16, isOutput=False)
    out_d = nc.declare_dram_parameter("part", [NWAY + 1, Q], f32, isOutput=True)

    with tile.TileContext(nc) as tc:
        with (
            tc.tile_pool(name="const", bufs=1) as cpool,
            tc.tile_pool(name="stage", bufs=1) as spool,
            tc.tile_pool(name="hpool", bufs=16) as hpool,
            tc.tile_pool(name="psum", bufs=8, space="PSUM") as ppool,
        ):
            # ---- inputs ----------------------------------------------
            blob_t = cpool.tile([128, NB], b16, name="blobt")
            qT_t = [spool.tile([128, Q], b16, name=f"qTt{i}") for i in range(2)]
            ones_t = cpool.tile([1, SP], b16, name="onest")
            nc.sync.dma_start(out=blob_t[:], in_=blob_d[:])
            for i in range(2):
                for h in range(2):
                    nc.sync.dma_start(
                        out=qT_t[i][:, Q // 2 * h : Q // 2 * (h + 1)],
                        in_=qT_d[128 * i : 128 * (i + 1), Q // 2 * h : Q // 2 * (h + 1)],
                    )
            nc.vector.memset(ones_t[:], 1.0)

            def w1a(dinb, doutb):
                o = OFF_W1A + 256 * dinb + 128 * doutb
                return blob_t[:, o : o + 128]

            def w1b(dinb, doutb):
                o = OFF_W1B + 256 * dinb + 128 * doutb
                return blob_t[:, o : o + 128]

            def sT(dinb):
                o = OFF_ST + SP * dinb
                return blob_t[:, o : o + SP]

            def w2col(db, r):
                o = OFF_W2C + 32 * (db * NR + r)
                return blob_t[:, o : o + 32]

            def ohm(half):
                o = OFF_OHM + (NWAY + 1) * half
                return blob_t[:, o : o + NWAY + 1]

            def b1row(db):
                o = OFF_B1 + 128 * db
                return blob_t[0:1, o : o + 128]

            # ---- spbT = W1b^T @ supportT + b1   [2][128, SP] f32 ------
            # b1 folds in as a K=1 rank-1 update (lhsT = b1 row, rhs =
            # ones): TensorScalarPtr has one sync-wait slot in its HW
            # struct, so a psum+bias add on DVE is not encodable here.
            spb_t = [cpool.tile([128, SP], f32, name=f"spb{i}") for i in range(2)]
            for db in range(2):
                sps = ppool.tile([128, QC], f32, tag="ps", name=f"sps{db}")
                nc.tensor.matmul(sps[:, :SP], w1b(0, db), sT(0), start=True, stop=False)
                nc.tensor.matmul(sps[:, :SP], w1b(1, db), sT(1), start=False, stop=False)
                nc.tensor.matmul(sps[:, :SP], b1row(db), ones_t[:], start=False, stop=True)
                nc.scalar.copy(out=spb_t[db][:], in_=sps[:, :SP])

            # ---- qpT = W1a^T @ queryT   [2][128, Q] bf16 --------------
            # copies on ACT: consumers' first ops spend their single
            # wait slot on the ACT sem once; later ops only wait on PE
            # for H-slot recycling.
            qpT_t = [spool.tile([128, Q], b16, name=f"qpT{i}") for i in range(2)]
            for db in range(2):
                for qc in range(NQC):
                    qps = ppool.tile([128, QC], f32, tag="ps", name=f"qps{db}{qc}")
                    nc.tensor.matmul(
                        qps[:], w1a(0, db), qT_t[0][:, QC * qc : QC * (qc + 1)],
                        start=True, stop=False,
                    )
                    nc.tensor.matmul(
                        qps[:], w1a(1, db), qT_t[1][:, QC * qc : QC * (qc + 1)],
                        start=False, stop=True,
                    )
                    nc.scalar.copy(
                        out=qpT_t[db][:, QC * qc : QC * (qc + 1)], in_=qps[:]
                    )

            # ---- main loop -------------------------------------------
            e_t = spool.tile([128, Q], b16, name="et")
            out_sb = spool.tile([NWAY + 1, Q], f32, name="outsb")
            fps = {}
            ts_idx = 0
            for half in range(2):
                scores_ps = [
                    ppool.tile([128, QC], f32, tag="ps", name=f"sc{half}{qc}")
                    for qc in range(NQC)
                ]
                for rh in range(NRH):
                    r = half * NRH + rh
                    h_tiles = {}
                    for j in range(4):
                        sl = 4 * r + j
                        for db in range(2):
                            # 58/22 DVE/ACT split by measured rates
                            # (~749 vs ~1990 ns).  GPSIMD ruled out:
                            # ~30us/op and SBUF-port contention slows
                            # DVE 8x.  Separate slot tags per producer
                            # engine keep every op at ONE cross-engine
                            # wait (short AC/TS structs have a single
                            # sync-wait slot).
                            use_act = (ts_idx * 21) % 80 < 21   # 21 of 80 ops
                            if use_act:
                                h = hpool.tile(
                                    [128, Q], b16, tag="Ha", bufs=6, name=f"h{sl}_{db}"
                                )
                                nc.scalar.activation(
                                    h[:], qpT_t[db][:], RELU,
                                    bias=spb_t[db][:, sl : sl + 1],
                                )
                            else:
                                h = hpool.tile(
                                    [128, Q], b16, tag="Hd", bufs=14, name=f"h{sl}_{db}"
                                )
                                nc.vector.tensor_scalar(
                                    out=h[:], in0=qpT_t[db][:],
                                    scalar1=spb_t[db][:, sl : sl + 1],
                                    scalar2=0.0, op0=ADD, op1=MAX,
                                )
                            ts_idx += 1
                            h_tiles[(j, db)] = h
                    for db in range(2):
                        for qc in range(NQC):
                            for j in range(4):
                                nc.tensor.matmul(
                                    scores_ps[qc][32 * j : 32 * j + 32, :],
                                    w2col(db, r),
                                    h_tiles[(j, db)][:, QC * qc : QC * (qc + 1)],
                                    start=(rh == 0 and db == 0),
                                    stop=(rh == NRH - 1 and db == 1),
                                    tile_position=(0, 32 * j),
                                    skip_group_check=True,
                                )
                # exp + partial-output matmul for this half; for half 0
                # this overlaps the second half of the main loop.
                for qc in range(NQC):
                    nc.scalar.activation(
                        e_t[:, QC * qc : QC * (qc + 1)], scores_ps[qc][:], EXP,
                    )
                    if half == 0:
                        fps[qc] = ppool.tile(
                            [NWAY + 1, QC], f32, tag="ps", name=f"fps{qc}"
                        )
                    nc.tensor.matmul(
                        fps[qc][:], ohm(half), e_t[:, QC * qc : QC * (qc + 1)],
                        start=(half == 0), stop=(half == 1),
                    )
                    if half == 1:
                        dst = out_sb[:, QC * qc : QC * (qc + 1)]
                        nc.vector.tensor_copy(out=dst, in_=fps[qc][:])
                        nc.sync.dma_start(
                            out=out_d[:, QC * qc : QC * (qc + 1)], in_=dst,
                        )

    nc.finalize()
    return nc


def _host_prep(inputs):
    """Host-side layout prep: transposes, dtype casts, one-hot tables.

    Returns the list of 8 per-core input dicts for the bass kernel.
    """
    q = np.ascontiguousarray(np.asarray(inputs["query_embeddings"], dtype=np.float32))
    s = np.ascontiguousarray(np.asarray(inputs["support_embeddings"], dtype=np.float32))
    lab = np.asarray(inputs["support_labels"]).astype(np.int64)
    W1 = np.asarray(inputs["W1"], dtype=np.float32)
    b1 = np.asarray(inputs["b1"], dtype=np.float32)
    W2 = np.asarray(inputs["W2"], dtype=np.float32)

    qT = np.ascontiguousarray(q.T).astype(bf16)            # [D, Q]
    sT_full = np.ascontiguousarray(s.T).astype(np.float32) # [D, S]

    blob0 = np.zeros((128, NB), dtype=np.float32)
    for dinb in range(2):
        blob0[:, OFF_W1A + 256 * dinb : OFF_W1A + 256 * (dinb + 1)] = W1[
            128 * dinb : 128 * (dinb + 1)
        ]
        blob0[:, OFF_W1B + 256 * dinb : OFF_W1B + 256 * (dinb + 1)] = W1[
            D + 128 * dinb : D + 128 * (dinb + 1)
        ]
    for db in range(2):
        blk = W2[128 * db : 128 * (db + 1)]
        for r in range(NR):
            blob0[:, OFF_W2C + 32 * (db * NR + r) + r] = blk
    blob0[0, OFF_B1 : OFF_B1 + D] = b1

    in_maps = []
    for c in range(N_CORES):
        lo = c * SP
        blob = blob0.copy()
        for dinb in range(2):
            blob[:, OFF_ST + SP * dinb : OFF_ST + SP * (dinb + 1)] = sT_full[
                128 * dinb : 128 * (dinb + 1), lo : lo + SP
            ]
        for sl in range(SP):
            half = sl // (SP // 2)
            row = 32 * (sl % 4) + sl // 4
            col = OFF_OHM + (NWAY + 1) * half
            blob[row, col + lab[lo + sl]] = 1.0
            blob[row, col + NWAY] = 1.0
        in_maps.append({"blob": blob.astype(bf16), "qT": qT})
    return in_maps


def _combine(parts):
    """Sum per-core partials and normalize -> [Q, NWAY] f32."""
    total = np.zeros((NWAY + 1, Q), dtype=np.float32)
    for p in parts:
        total += np.asarray(p, dtype=np.float32)
    return np.ascontiguousarray((total[:NWAY] / total[NWAY : NWAY + 1]).T)


def get_nc():
    global _compiled
    if _compiled is None:
        _compiled = _build_nc()
    return _compiled


def kernel(**inputs) -> np.ndarray:
    from concourse.bass_utils import run_bass_kernel_spmd

    nc = get_nc()
    in_maps = _host_prep(inputs)
    res = run_bass_kernel_spmd(nc, in_maps, list(range(N_CORES)))
    return _combine([res.results[c]["part"] for c in range(N_CORES)])


# revision 28
# speedup vs baseline: 1.0317x; 1.0317x over previous
"""Trainium2 Bass kernel for a Matching Network attention head.

Reference computation (see problem statement):
    q_proj = query @ W1[:D]                       # [Q, D]
    s_proj = support @ W1[D:]                     # [S, D]
    hidden = relu(q_proj[:,None,:] + s_proj[None,:,:] + b1)   # [Q, S, D]
    scores = einsum('qsd,d->qs', hidden, W2) + b2
    weights = softmax(scores, axis=1)
    logits  = weights @ onehot(support_labels)    # [Q, n_way]

Sharding strategy (8 cores): shard the SUPPORT set (40 of 320 rows per
core) and replicate the queries.  Each core produces the *unnormalized*
partial numerators and denominator of the softmax-weighted average:

    part[w, q]  = sum_{s in shard} exp(score[s,q]) * onehot[s,w]   (w < 20)
    part[20, q] = sum_{s in shard} exp(score[s,q])

The host sums the partials over cores and divides - softmax over the
full support set falls out exactly (b2 is a constant shift over s and
cancels in the softmax, so it is dropped).  exp() is computed without a
max-subtraction: scores are ~N(0, 0.7) for this problem so fp32 exp is
safe and exact.

Per-core device program:
  - one const "blob" DMA + two queryT DMAs (bf16, host-prepped layout)
  - qpT[dout, q]  = W1a^T @ queryT      (PE, bf16, fp32 psum)
  - spbT[dout, s] = W1b^T @ supportT + b1 (b1 folded in as a K=1 matmul)
  - For each s: H = relu(qpT + spbT[:, s]) as a single fused
    tensor_scalar(add, max) on DVE (bf16 in/out -> 4x mode, ~749 ns) or
    an activation(Relu, bias) on ACT (~1990 ns); 58/22 split so both
    engines finish together.
  - scores[s, q] = sum_d W2[d] * H[d, q] via one-hot-column matmuls:
    lhsT is [128, 32] with W2's d-block in column (s//4), output goes to
    psum partitions [32*(s%4) .. +32).  tile_position=(0, 32*j) makes 4
    consecutive matmuls run concurrently in distinct 32-column groups of
    the PE array.  s is split into two halves with separate psum bank
    sets so exp/matmul of the first half overlaps the second half.
  - E = exp(scores) on ACT (psum -> sbuf, bf16)
  - part += [onehot_half | ones_half]^T @ E on PE, copied out [21, Q].
"""

import numpy as np
import ml_dtypes

bf16 = ml_dtypes.bfloat16

N_CORES = 8
Q, D, S, NWAY = 2048, 256, 320, 20
SP = S // N_CORES          # 40 support rows per core
NQC = 4                    # q chunks of 512 (one psum bank each)
QC = Q // NQC
NR = SP // 4               # 10 rounds of 4 concurrent s-values
NRH = NR // 2              # rounds per half (5)

# const-blob column layout (bf16, [128, NB])
OFF_W1A = 0                # [128, 256] x2 (din block major)
OFF_W1B = 512
OFF_ST = 1024              # [128, 40] x2
OFF_W2C = 1104             # [128, 640]: 2 dblk x 10 rounds x [128, 32]
OFF_OHM = 1744             # [128, 21] x2 (ohmA | ohmB)
OFF_B1 = 1786              # [row0 = b1, 256 cols]
NB = 2042

_compiled = None


def _build_nc():
    import concourse.tile as tile
    from concourse import mybir
    from concourse.bacc import Bacc

    f32 = mybir.dt.float32
    b16 = mybir.dt.bfloat16
    RELU = mybir.ActivationFunctionType.Relu
    EXP = mybir.ActivationFunctionType.Exp
    ADD = mybir.AluOpType.add
    MAX = mybir.AluOpType.max

    nc = Bacc()
    blob_d = nc.declare_dram_parameter("blob", [128, NB], b16, isOutput=False)
    qT_d = nc.declare_dram_parameter("qT", [D, Q], b16, isOutput=False)
    out_d = nc.declare_dram_parameter("part", [NWAY + 1, Q], f32, isOutput=True)

    with tile.TileContext(nc) as tc:
        with (
            tc.tile_pool(name="const", bufs=1) as cpool,
            tc.tile_pool(name="stage", bufs=1) as spool,
            tc.tile_pool(name="hpool", bufs=16) as hpool,
            tc.tile_pool(name="psum", bufs=8, space="PSUM") as ppool,
        ):
            # ---- inputs ----------------------------------------------
            blob_t = cpool.tile([128, NB], b16, name="blobt")
            qT_t = [spool.tile([128, Q], b16, name=f"qTt{i}") for i in range(2)]
            ones_t = cpool.tile([1, SP], b16, name="onest")
            nc.sync.dma_start(out=blob_t[:], in_=blob_d[:])
            # q-half 0 for both din blocks first: the first qpT matmul
            # (q chunk 0) only needs these two transfers.
            for h in range(2):
                for i in range(2):
                    nc.sync.dma_start(
                        out=qT_t[i][:, Q // 2 * h : Q // 2 * (h + 1)],
                        in_=qT_d[128 * i : 128 * (i + 1), Q // 2 * h : Q // 2 * (h + 1)],
                    )
            nc.vector.memset(ones_t[:], 1.0)

            def w1a(dinb, doutb):
                o = OFF_W1A + 256 * dinb + 128 * doutb
                return blob_t[:, o : o + 128]

            def w1b(dinb, doutb):
                o = OFF_W1B + 256 * dinb + 128 * doutb
                return blob_t[:, o : o + 128]

            def sT(dinb):
                o = OFF_ST + SP * dinb
                return blob_t[:, o : o + SP]

            def w2col(db, r):
                o = OFF_W2C + 32 * (db * NR + r)
                return blob_t[:, o : o + 32]

            def ohm(half):
                o = OFF_OHM + (NWAY + 1) * half
                return blob_t[:, o : o + NWAY + 1]

            def b1row(db):
                o = OFF_B1 + 128 * db
                return blob_t[0:1, o : o + 128]

            # ---- spbT = W1b^T @ supportT + b1   [2][128, SP] f32 ------
            # b1 folds in as a K=1 rank-1 update (lhsT = b1 row, rhs =
            # ones): TensorScalarPtr has one sync-wait slot in its HW
            # struct, so a psum+bias add on DVE is not encodable here.
            spb_t = [cpool.tile([128, SP], f32, name=f"spb{i}") for i in range(2)]
            for db in range(2):
                sps = ppool.tile([128, QC], f32, tag="ps", name=f"sps{db}")
                nc.tensor.matmul(sps[:, :SP], w1b(0, db), sT(0), start=True, stop=False)
                nc.tensor.matmul(sps[:, :SP], w1b(1, db), sT(1), start=False, stop=False)
                nc.tensor.matmul(sps[:, :SP], b1row(db), ones_t[:], start=False, stop=True)
                nc.scalar.copy(out=spb_t[db][:], in_=sps[:, :SP])

            # ---- qpT = W1a^T @ queryT   [2][128, Q] bf16 --------------
            # copies on ACT: consumers' first ops spend their single
            # wait slot on the ACT sem once; later ops only wait on PE
            # for H-slot recycling.
            qpT_t = [spool.tile([128, Q], b16, name=f"qpT{i}") for i in range(2)]
            for db in range(2):
                for qc in range(NQC):
                    qps = ppool.tile([128, QC], f32, tag="ps", name=f"qps{db}{qc}")
                    nc.tensor.matmul(
                        qps[:], w1a(0, db), qT_t[0][:, QC * qc : QC * (qc + 1)],
                        start=True, stop=False,
                    )
                    nc.tensor.matmul(
                        qps[:], w1a(1, db), qT_t[1][:, QC * qc : QC * (qc + 1)],
                        start=False, stop=True,
                    )
                    nc.scalar.copy(
                        out=qpT_t[db][:, QC * qc : QC * (qc + 1)], in_=qps[:]
                    )

            # ---- main loop -------------------------------------------
            # 62/18 DVE/ACT split by measured rates (~749 vs ~1990 ns
            # per [128, 2048] op).  GPSIMD ruled out: ~30us/op and
            # SBUF-port contention slows DVE 8x.  Separate slot tags per
            # producer engine keep every op at ONE cross-engine wait
            # (short AC/TS structs have a single sync-wait slot).  ACT
            # gets no ops in the last round so exp can start while the
            # last scores matmuls run.  The first two rounds emit
            # per-q-chunk ops so the pipeline starts as soon as the
            # first qpT chunk is ready instead of waiting for all of it.
            e_t = spool.tile([128, Q], b16, name="et")
            out_sb = spool.tile([NWAY + 1, Q], f32, name="outsb")
            scores_ps = [
                ppool.tile([128, QC], f32, tag="ps", name=f"sc{qc}")
                for qc in range(NQC)
            ]
            ts_idx = 0
            for r in range(NR):
                h_tiles = {}
                for j in range(4):
                    sl = 4 * r + j
                    for db in range(2):
                        use_act = r < NR - 1 and (ts_idx * 18) % 72 < 18
                        if use_act:
                            h = hpool.tile(
                                [128, Q], b16, tag="Ha", bufs=8, name=f"h{sl}_{db}"
                            )
                            nc.scalar.activation(
                                h[:], qpT_t[db][:], RELU,
                                bias=spb_t[db][:, sl : sl + 1],
                            )
                        else:
                            h = hpool.tile(
                                [128, Q], b16, tag="Hd", bufs=26, name=f"h{sl}_{db}"
                            )
                            if r < 2:
                                for qc in range(NQC):
                                    nc.vector.tensor_scalar(
                                        out=h[:, QC * qc : QC * (qc + 1)],
                                        in0=qpT_t[db][:, QC * qc : QC * (qc + 1)],
                                        scalar1=spb_t[db][:, sl : sl + 1],
                                        scalar2=0.0, op0=ADD, op1=MAX,
                                    )
                            else:
                                nc.vector.tensor_scalar(
                                    out=h[:], in0=qpT_t[db][:],
                                    scalar1=spb_t[db][:, sl : sl + 1],
                                    scalar2=0.0, op0=ADD, op1=MAX,
                                )
                        if r < NR - 1:
                            ts_idx += 1
                        h_tiles[(j, db)] = h
                for db in range(2):
                    for qc in range(NQC):
                        for j in range(4):
                            nc.tensor.matmul(
                                scores_ps[qc][32 * j : 32 * j + 32, :],
                                w2col(db, r),
                                h_tiles[(j, db)][:, QC * qc : QC * (qc + 1)],
                                start=(r == 0 and db == 0),
                                stop=(r == NR - 1 and db == 1),
                                tile_position=(0, 32 * j),
                                skip_group_check=True,
                            )

            # ---- tail, pipelined per q-chunk -------------------------
            for qc in range(NQC):
                nc.scalar.activation(
                    e_t[:, QC * qc : QC * (qc + 1)], scores_ps[qc][:], EXP,
                )
                fps = ppool.tile([NWAY + 1, QC], f32, tag="ps", name=f"fps{qc}")
                nc.tensor.matmul(
                    fps[:], ohm(0), e_t[:, QC * qc : QC * (qc + 1)],
                    start=True, stop=True,
                )
                dst = out_sb[:, QC * qc : QC * (qc + 1)]
                nc.vector.tensor_copy(out=dst, in_=fps[:])
                nc.sync.dma_start(out=out_d[:, QC * qc : QC * (qc + 1)], in_=dst)

    nc.finalize()
    return nc


def _host_prep(inputs):
    """Host-side layout prep: transposes, dtype casts, one-hot tables.

    Returns the list of 8 per-core input dicts for the bass kernel.
    """
    q = np.ascontiguousarray(np.asarray(inputs["query_embeddings"], dtype=np.float32))
    s = np.ascontiguousarray(np.asarray(inputs["support_embeddings"], dtype=np.float32))
    lab = np.asarray(inputs["support_labels"]).astype(np.int64)
    W1 = np.asarray(inputs["W1"], dtype=np.float32)
    b1 = np.asarray(inputs["b1"], dtype=np.float32)
    W2 = np.asarray(inputs["W2"], dtype=np.float32)

    qT = np.ascontiguousarray(q.T).astype(bf16)            # [D, Q]
    sT_full = np.ascontiguousarray(s.T).astype(np.float32) # [D, S]

    blob0 = np.zeros((128, NB), dtype=np.float32)
    for dinb in range(2):
        blob0[:, OFF_W1A + 256 * dinb : OFF_W1A + 256 * (dinb + 1)] = W1[
            128 * dinb : 128 * (dinb + 1)
        ]
        blob0[:, OFF_W1B + 256 * dinb : OFF_W1B + 256 * (dinb + 1)] = W1[
            D + 128 * dinb : D + 128 * (dinb + 1)
        ]
    for db in range(2):
        blk = W2[128 * db : 128 * (db + 1)]
        for r in range(NR):
            blob0[:, OFF_W2C + 32 * (db * NR + r) + r] = blk
    blob0[0, OFF_B1 : OFF_B1 + D] = b1

    in_maps = []
    for c in range(N_CORES):
        lo = c * SP
        blob = blob0.copy()
        for dinb in range(2):
            blob[:, OFF_ST + SP * dinb : OFF_ST + SP * (dinb + 1)] = sT_full[
                128 * dinb : 128 * (dinb + 1), lo : lo + SP
            ]
        for sl in range(SP):
            row = 32 * (sl % 4) + sl // 4
            blob[row, OFF_OHM + lab[lo + sl]] = 1.0
            blob[row, OFF_OHM + NWAY] = 1.0
        in_maps.append({"blob": blob.astype(bf16), "qT": qT})
    return in_maps


def _combine(parts):
    """Sum per-core partials and normalize -> [Q, NWAY] f32."""
    total = np.zeros((NWAY + 1, Q), dtype=np.float32)
    for p in parts:
        total += np.asarray(p, dtype=np.float32)
    return np.ascontiguousarray((total[:NWAY] / total[NWAY : NWAY + 1]).T)


def get_nc():
    global _compiled
    if _compiled is None:
        _compiled = _build_nc()
    return _compiled


def kernel(**inputs) -> np.ndarray:
    from concourse.bass_utils import run_bass_kernel_spmd

    nc = get_nc()
    in_maps = _host_prep(inputs)
    res = run_bass_kernel_spmd(nc, in_maps, list(range(N_CORES)))
    return _combine([res.results[c]["part"] for c in range(N_CORES)])


# revision 31
# speedup vs baseline: 1.0387x; 1.0068x over previous
"""Trainium2 Bass kernel for a Matching Network attention head.

Reference computation (see problem statement):
    q_proj = query @ W1[:D]                       # [Q, D]
    s_proj = support @ W1[D:]                     # [S, D]
    hidden = relu(q_proj[:,None,:] + s_proj[None,:,:] + b1)   # [Q, S, D]
    scores = einsum('qsd,d->qs', hidden, W2) + b2
    weights = softmax(scores, axis=1)
    logits  = weights @ onehot(support_labels)    # [Q, n_way]

Sharding strategy (8 cores): shard the SUPPORT set (40 of 320 rows per
core) and replicate the queries.  Each core produces the *unnormalized*
partial numerators and denominator of the softmax-weighted average:

    part[w, q]  = sum_{s in shard} exp(score[s,q]) * onehot[s,w]   (w < 20)
    part[20, q] = sum_{s in shard} exp(score[s,q])

The host sums the partials over cores and divides - softmax over the
full support set falls out exactly (b2 is a constant shift over s and
cancels in the softmax, so it is dropped).  exp() is computed without a
max-subtraction: scores are ~N(0, 0.7) for this problem so fp32 exp is
safe and exact.

Per-core device program:
  - one const "blob" DMA + two queryT DMAs (bf16, host-prepped layout)
  - qpT[dout, q]  = W1a^T @ queryT      (PE, bf16, fp32 psum)
  - spbT[dout, s] = W1b^T @ supportT + b1 (b1 folded in as a K=1 matmul)
  - For each s: H = relu(qpT + spbT[:, s]) as a single fused
    tensor_scalar(add, max) on DVE (bf16 in/out -> 4x mode, ~749 ns) or
    an activation(Relu, bias) on ACT (~1990 ns); 58/22 split so both
    engines finish together.
  - scores[s, q] = sum_d W2[d] * H[d, q] via one-hot-column matmuls:
    lhsT is [128, 32] with W2's d-block in column (s//4), output goes to
    psum partitions [32*(s%4) .. +32).  tile_position=(0, 32*j) makes 4
    consecutive matmuls run concurrently in distinct 32-column groups of
    the PE array.  s is split into two halves with separate psum bank
    sets so exp/matmul of the first half overlaps the second half.
  - E = exp(scores) on ACT (psum -> sbuf, bf16)
  - part += [onehot_half | ones_half]^T @ E on PE, copied out [21, Q].
"""

import numpy as np
import ml_dtypes

bf16 = ml_dtypes.bfloat16

N_CORES = 8
Q, D, S, NWAY = 2048, 256, 320, 20
SP = S // N_CORES          # 40 support rows per core
NQC = 4                    # q chunks of 512 (one psum bank each)
QC = Q // NQC
NR = SP // 4               # 10 rounds of 4 concurrent s-values
NRH = NR // 2              # rounds per half (5)

# const-blob column layout (bf16, [128, NB])
OFF_W1A = 0                # [128, 256] x2 (din block major)
OFF_W1B = 512
OFF_ST = 1024              # [128, 40] x2
OFF_W2C = 1104             # [128, 640]: 2 dblk x 10 rounds x [128, 32]
OFF_OHM = 1744             # [128, 21] x2 (ohmA | ohmB)
OFF_B1 = 1786              # [row0 = b1, 256 cols]
NB = 2042

_compiled = None


def _build_nc():
    import concourse.tile as tile
    from concourse import mybir
    from concourse.bacc import Bacc

    f32 = mybir.dt.float32
    b16 = mybir.dt.bfloat16
    RELU = mybir.ActivationFunctionType.Relu
    EXP = mybir.ActivationFunctionType.Exp
    ADD = mybir.AluOpType.add
    MAX = mybir.AluOpType.max

    nc = Bacc()
    blob_d = nc.declare_dram_parameter("blob", [128, NB], b16, isOutput=False)
    qT_d = nc.declare_dram_parameter("qT", [D, Q], b16, isOutput=False)
    out_d = nc.declare_dram_parameter("part", [NWAY + 1, Q], f32, isOutput=True)

    with tile.TileContext(nc) as tc:
        with (
            tc.tile_pool(name="const", bufs=1) as cpool,
            tc.tile_pool(name="stage", bufs=1) as spool,
            tc.tile_pool(name="hpool", bufs=16) as hpool,
            tc.tile_pool(name="psum", bufs=8, space="PSUM") as ppool,
        ):
            # ---- inputs ----------------------------------------------
            blob_t = cpool.tile([128, NB], b16, name="blobt")
            qT_t = [spool.tile([128, Q], b16, name=f"qTt{i}") for i in range(2)]
            ones_t = cpool.tile([1, SP], b16, name="onest")
            # blob on the ACT HWDGE ring, qT on the SP ring: the first
            # PE matmuls depend only on blob and must not FIFO behind
            # the 1MB of qT transfers.
            nc.scalar.dma_start(out=blob_t[:], in_=blob_d[:])
            # q-half 0 for both din blocks first: the first qpT matmul
            # (q chunk 0) only needs these two transfers.
            for h in range(2):
                for i in range(2):
                    nc.sync.dma_start(
                        out=qT_t[i][:, Q // 2 * h : Q // 2 * (h + 1)],
                        in_=qT_d[128 * i : 128 * (i + 1), Q // 2 * h : Q // 2 * (h + 1)],
                    )
            nc.vector.memset(ones_t[:], 1.0)

            def w1a(dinb, doutb):
                o = OFF_W1A + 256 * dinb + 128 * doutb
                return blob_t[:, o : o + 128]

            def w1b(dinb, doutb):
                o = OFF_W1B + 256 * dinb + 128 * doutb
                return blob_t[:, o : o + 128]

            def sT(dinb):
                o = OFF_ST + SP * dinb
                return blob_t[:, o : o + SP]

            def w2col(db, r):
                o = OFF_W2C + 32 * (db * NR + r)
                return blob_t[:, o : o + 32]

            def ohm(half):
                o = OFF_OHM + (NWAY + 1) * half
                return blob_t[:, o : o + NWAY + 1]

            def b1row(db):
                o = OFF_B1 + 128 * db
                return blob_t[0:1, o : o + 128]

            # ---- spbT = W1b^T @ supportT + b1   [2][128, SP] f32 ------
            # b1 folds in as a K=1 rank-1 update (lhsT = b1 row, rhs =
            # ones): TensorScalarPtr has one sync-wait slot in its HW
            # struct, so a psum+bias add on DVE is not encodable here.
            spb_t = [cpool.tile([128, SP], f32, name=f"spb{i}") for i in range(2)]
            for db in range(2):
                sps = ppool.tile([128, QC], f32, tag="ps", name=f"sps{db}")
                nc.tensor.matmul(sps[:, :SP], w1b(0, db), sT(0), start=True, stop=False)
                nc.tensor.matmul(sps[:, :SP], w1b(1, db), sT(1), start=False, stop=False)
                nc.tensor.matmul(sps[:, :SP], b1row(db), ones_t[:], start=False, stop=True)
                nc.scalar.copy(out=spb_t[db][:], in_=sps[:, :SP])

            # ---- qpT = W1a^T @ queryT   [2][128, Q] bf16 --------------
            # copies on ACT: consumers' first ops spend their single
            # wait slot on the ACT sem once; later ops only wait on PE
            # for H-slot recycling.
            # db0 copies on DVE, db1 on ACT: two parallel psum->sbuf
            # chains, and each engine's main-loop ops read the qpT half
            # it produced itself where possible (fewer cross waits).
            qpT_t = [spool.tile([128, Q], b16, name=f"qpT{i}") for i in range(2)]
            for db in range(2):
                for qc in range(NQC):
                    qps = ppool.tile([128, QC], f32, tag="ps", name=f"qps{db}{qc}")
                    nc.tensor.matmul(
                        qps[:], w1a(0, db), qT_t[0][:, QC * qc : QC * (qc + 1)],
                        start=True, stop=False,
                    )
                    nc.tensor.matmul(
                        qps[:], w1a(1, db), qT_t[1][:, QC * qc : QC * (qc + 1)],
                        start=False, stop=True,
                    )
                    dst = qpT_t[db][:, QC * qc : QC * (qc + 1)]
                    if db == 0:
                        nc.vector.tensor_copy(out=dst, in_=qps[:])
                    else:
                        nc.scalar.copy(out=dst, in_=qps[:])

            # ---- main loop -------------------------------------------
            # 62/18 DVE/ACT split by measured rates (~749 vs ~1990 ns
            # per [128, 2048] op).  GPSIMD ruled out: ~30us/op and
            # SBUF-port contention slows DVE 8x.  Separate slot tags per
            # producer engine keep every op at ONE cross-engine wait
            # (short AC/TS structs have a single sync-wait slot).  ACT
            # gets no ops in the last round so exp can start while the
            # last scores matmuls run.  The first two rounds emit
            # per-q-chunk ops so the pipeline starts as soon as the
            # first qpT chunk is ready instead of waiting for all of it.
            e_t = spool.tile([128, Q], b16, name="et")
            out_sb = spool.tile([NWAY + 1, Q], f32, name="outsb")
            scores_ps = [
                ppool.tile([128, QC], f32, tag="ps", name=f"sc{qc}")
                for qc in range(NQC)
            ]
            ts_idx = 0
            for r in range(NR):
                h_tiles = {}
                for j in range(4):
                    sl = 4 * r + j
                    for db in range(2):
                        use_act = r < NR - 1 and (ts_idx * 18) % 72 < 18
                        if use_act:
                            h = hpool.tile(
                                [128, Q], b16, tag="Ha", bufs=8, name=f"h{sl}_{db}"
                            )
                            nc.scalar.activation(
                                h[:], qpT_t[db][:], RELU,
                                bias=spb_t[db][:, sl : sl + 1],
                            )
                        else:
                            h = hpool.tile(
                                [128, Q], b16, tag="Hd", bufs=26, name=f"h{sl}_{db}"
                            )
                            if r < 2:
                                for qc in range(NQC):
                                    nc.vector.tensor_scalar(
                                        out=h[:, QC * qc : QC * (qc + 1)],
                                        in0=qpT_t[db][:, QC * qc : QC * (qc + 1)],
                                        scalar1=spb_t[db][:, sl : sl + 1],
                                        scalar2=0.0, op0=ADD, op1=MAX,
                                    )
                            else:
                                nc.vector.tensor_scalar(
                                    out=h[:], in0=qpT_t[db][:],
                                    scalar1=spb_t[db][:, sl : sl + 1],
                                    scalar2=0.0, op0=ADD, op1=MAX,
                                )
                        if r < NR - 1:
                            ts_idx += 1
                        h_tiles[(j, db)] = h
                for db in range(2):
                    for qc in range(NQC):
                        for j in range(4):
                            nc.tensor.matmul(
                                scores_ps[qc][32 * j : 32 * j + 32, :],
                                w2col(db, r),
                                h_tiles[(j, db)][:, QC * qc : QC * (qc + 1)],
                                start=(r == 0 and db == 0),
                                stop=(r == NR - 1 and db == 1),
                                tile_position=(0, 32 * j),
                                skip_group_check=True,
                            )

            # ---- tail, pipelined per q-chunk -------------------------
            for qc in range(NQC):
                nc.scalar.activation(
                    e_t[:, QC * qc : QC * (qc + 1)], scores_ps[qc][:], EXP,
                )
                fps = ppool.tile([NWAY + 1, QC], f32, tag="ps", name=f"fps{qc}")
                nc.tensor.matmul(
                    fps[:], ohm(0), e_t[:, QC * qc : QC * (qc + 1)],
                    start=True, stop=True,
                )
                dst = out_sb[:, QC * qc : QC * (qc + 1)]
                nc.vector.tensor_copy(out=dst, in_=fps[:])
                nc.sync.dma_start(out=out_d[:, QC * qc : QC * (qc + 1)], in_=dst)

    nc.finalize()
    return nc


def _host_prep(inputs):
    """Host-side layout prep: transposes, dtype casts, one-hot tables.

    Returns the list of 8 per-core input dicts for the bass kernel.
    """
    q = np.ascontiguousarray(np.asarray(inputs["query_embeddings"], dtype=np.float32))
    s = np.ascontiguousarray(np.asarray(inputs["support_embeddings"], dtype=np.float32))
    lab = np.asarray(inputs["support_labels"]).astype(np.int64)
    W1 = np.asarray(inputs["W1"], dtype=np.float32)
    b1 = np.asarray(inputs["b1"], dtype=np.float32)
    W2 = np.asarray(inputs["W2"], dtype=np.float32)

    qT = np.ascontiguousarray(q.T).astype(bf16)            # [D, Q]
    sT_full = np.ascontiguousarray(s.T).astype(np.float32) # [D, S]

    blob0 = np.zeros((128, NB), dtype=np.float32)
    for dinb in range(2):
        blob0[:, OFF_W1A + 256 * dinb : OFF_W1A + 256 * (dinb + 1)] = W1[
            128 * dinb : 128 * (dinb + 1)
        ]
        blob0[:, OFF_W1B + 256 * dinb : OFF_W1B + 256 * (dinb + 1)] = W1[
            D + 128 * dinb : D + 128 * (dinb + 1)
        ]
    for db in range(2):
        blk = W2[128 * db : 128 * (db + 1)]
        for r in range(NR):
            blob0[:, OFF_W2C + 32 * (db * NR + r) + r] = blk
    blob0[0, OFF_B1 : OFF_B1 + D] = b1

    in_maps = []
    for c in range(N_CORES):
        lo = c * SP
        blob = blob0.copy()
        for dinb in range(2):
            blob[:, OFF_ST + SP * dinb : OFF_ST + SP * (dinb + 1)] = sT_full[
                128 * dinb : 128 * (dinb + 1), lo : lo + SP
            ]
        for sl in range(SP):
            row = 32 * (sl % 4) + sl // 4
            blob[row, OFF_OHM + lab[lo + sl]] = 1.0
            blob[row, OFF_OHM + NWAY] = 1.0
        in_maps.append({"blob": blob.astype(bf16), "qT": qT})
    return in_maps


def _combine(parts):
    """Sum per-core partials and normalize -> [Q, NWAY] f32."""
    total = np.zeros((NWAY + 1, Q), dtype=np.float32)
    for p in parts:
        total += np.asarray(p, dtype=np.float32)
    return np.ascontiguousarray((total[:NWAY] / total[NWAY : NWAY + 1]).T)


def get_nc():
    global _compiled
    if _compiled is None:
        _compiled = _build_nc()
    return _compiled


def kernel(**inputs) -> np.ndarray:
    from concourse.bass_utils import run_bass_kernel_spmd

    nc = get_nc()
    in_maps = _host_prep(inputs)
    res = run_bass_kernel_spmd(nc, in_maps, list(range(N_CORES)))
    return _combine([res.results[c]["part"] for c in range(N_CORES)])
